# revision 22
# baseline (speedup 1.0000x reference)
"""Trainium2 Bass kernel for nn_DecoderWithPositionLayer (8 NeuronCores).

Sharding: 2 groups x 4 cores; group g owns batch g (256 tokens).
Within a group (rank s = core % 4):
  - FFN mm1 hidden-split 4-way, relu h AllGathered (bf16), mm2 output-split.
  - Attention split by query tokens (64 q/core). The relative-position bias
    is reassociated:  bias[q,k] = sum_f qp[q,f] rp[q,k,f],
                      qp[h,q,f] = sum_d q[h,q,d] pos_w[f, h*64+d]
    (pos_b adds a row-constant to logits -> cancels in softmax; dropped).
    The causal mask rides as an extra contraction row of host-transposed rp.
  - q-side activations are exchanged with AllToAll so every core's q slice
    lands at a fixed DRAM address (no rank-dependent addressing on device);
    k/v sides use AllGather.
Activations stay feature-major [feature, token]; the only transposes are
DMA-xbar bf16 transposes for attention V and softmax W. Matmul operands are
bf16 (host-cast weights) with f32 PSUM accumulation; LN/softmax math f32.
"""

import contextlib
import numpy as np
import ml_dtypes

import concourse.bass as bass
import concourse.bacc as bacc_mod
import concourse.tile as tile
from concourse import mybir
from concourse.bass_utils import run_bass_kernel_spmd

BF16 = ml_dtypes.bfloat16
FP32 = mybir.dt.float32
BF = mybir.dt.bfloat16
FR = mybir.dt.float32r

B, S, D, F, HID, H = 2, 256, 1024, 64, 4096, 16
DIM = D // H
G = 4
TLOC = S // G          # 64
HIDL = HID // G        # 1024
P = 128
EPS = 1e-3
NEG = -1e9
OUTD = [3 * D, D, 2 * D, D]
OUTD_S = [o // G for o in OUTD]    # 768, 256, 512, 256
RG = [[0, 1, 2, 3], [4, 5, 6, 7]]

_CACHE = {}


# ------------------------------------------------------------------ host prep
def _prep_in_maps(inp):
    f32 = np.float32
    qT = np.ascontiguousarray(np.transpose(np.asarray(inp["queries"], f32), (0, 2, 1)))
    vT = np.ascontiguousarray(np.transpose(np.asarray(inp["values"], f32), (0, 2, 1)))
    pos_wT = np.ascontiguousarray(np.asarray(inp["pos_w"], f32).T).astype(BF16)

    rp = np.asarray(inp["relative_positions"], f32)
    rpT = np.transpose(rp, (0, 1, 3, 2))                       # [B,S,F,S]
    mask = np.where(np.arange(S)[None, :] <= np.arange(S)[:, None], 0.0, NEG)
    rpT_ext = np.concatenate(
        [rpT, np.broadcast_to(mask[None, :, None, :], (B, S, 1, S))], axis=2
    ).astype(BF16)                                             # [B,S,F+1,S]

    blocks = []
    for i in range(4):
        p = f"b{i}_"
        g = np.asarray(inp[p + "ln_g"], f32)
        be = np.asarray(inp[p + "ln_b"], f32)
        w1 = np.asarray(inp[p + "w1"], f32)
        b1 = np.asarray(inp[p + "b1"], f32)
        w2 = np.asarray(inp[p + "w2"], f32)
        b2 = np.asarray(inp[p + "b2"], f32)
        blocks.append((g[:, None] * w1, b1 + be @ w1, w2, b2))

    in_maps = []
    for c in range(8):
        g, s = c // G, c % G
        m = {
            "xT": qT[g],
            "vT": vT[g],
            "xTslice": np.ascontiguousarray(qT[g][:, s * TLOC:(s + 1) * TLOC]),
            "pos_wT": pos_wT,
            "rpT_ext": np.ascontiguousarray(rpT_ext[g, s * TLOC:(s + 1) * TLOC]),
            "rankhot": np.ascontiguousarray(
                np.broadcast_to(np.eye(G, dtype=f32)[s], (P, G))),
        }
        for i, (w1f, b1f, w2, b2) in enumerate(blocks):
            w1s = np.ascontiguousarray(w1f[:, s * HIDL:(s + 1) * HIDL])
            m[f"w1_{i}"] = w1s.astype(BF16)
            m[f"w1sum_{i}"] = w1s.sum(axis=0, keepdims=True).astype(BF16)
            m[f"b1_{i}"] = b1f[s * HIDL:(s + 1) * HIDL].astype(f32)
            o = OUTD_S[i]
            m[f"w2_{i}"] = np.ascontiguousarray(w2[:, s * o:(s + 1) * o]).astype(BF16)
            m[f"b2_{i}"] = b2[s * o:(s + 1) * o].astype(f32)
        in_maps.append(m)
    return in_maps


# --------------------------------------------------------------- device build
def _build_nc():
    nc = bacc_mod.Bacc(num_devices=8)
    io = {}
    io["xT"] = nc.declare_dram_parameter("xT", [D, S], FP32, False)
    io["vT"] = nc.declare_dram_parameter("vT", [D, S], FP32, False)
    io["xTslice"] = nc.declare_dram_parameter("xTslice", [D, TLOC], FP32, False)
    io["rankhot"] = nc.declare_dram_parameter("rankhot", [P, G], FP32, False)
    io["pos_wT"] = nc.declare_dram_parameter("pos_wT", [D, F], BF, False)
    io["rpT_ext"] = nc.declare_dram_parameter("rpT_ext", [TLOC, F + 1, S], BF, False)
    for i in range(4):
        io[f"w1_{i}"] = nc.declare_dram_parameter(f"w1_{i}", [D, HIDL], BF, False)
        io[f"w1sum_{i}"] = nc.declare_dram_parameter(f"w1sum_{i}", [1, HIDL], BF, False)
        io[f"b1_{i}"] = nc.declare_dram_parameter(f"b1_{i}", [HIDL], FP32, False)
        io[f"w2_{i}"] = nc.declare_dram_parameter(f"w2_{i}", [HID, OUTD_S[i]], BF, False)
        io[f"b2_{i}"] = nc.declare_dram_parameter(f"b2_{i}", [OUTD_S[i]], FP32, False)
    io["x2T_out"] = nc.declare_dram_parameter("x2T_out", [D, TLOC], FP32, True)
    io["o3T_out"] = nc.declare_dram_parameter("o3T_out", [OUTD_S[3], S], FP32, True)

    with tile.TileContext(nc) as tc:
        _program(nc, tc, io)
    nc.finalize()
    return nc


def _ap(t, offset, pattern):
    tensor = t.tensor if isinstance(t, bass.AP) else t.ap().tensor
    return bass.AP(tensor=tensor, offset=offset, ap=pattern)


def _program(nc, tc, io):
    import os
    KMODE = int(os.environ.get("KMODE", "8"))
    NOTR = bool(int(os.environ.get("NOTR", "0")))
    ctx = contextlib.ExitStack()
    sb = ctx.enter_context(tc.tile_pool(name="sb", bufs=2))
    consts = ctx.enter_context(tc.tile_pool(name="consts", bufs=1))
    psum = ctx.enter_context(tc.tile_pool(name="psum", bufs=2, space="PSUM"))
    dram = ctx.enter_context(tc.tile_pool(name="dram", bufs=1, space="DRAM"))

    sync, vec, act, pe, gps = nc.sync, nc.vector, nc.scalar, nc.tensor, nc.gpsimd
    AF = mybir.ActivationFunctionType
    ALU = mybir.AluOpType

    ones_col = consts.tile([P, 1], BF, tag="ones_col")
    vec.memset(ones_col, 1.0)
    ones_row = consts.tile([1, P], BF, tag="ones_row")
    vec.memset(ones_row, 1.0)

    # pos_w^T head-chunked at partition base 0: [64(d), 16(h), 64(f)]
    poswT = consts.tile([DIM, H, F], BF, tag="poswT")
    sync.dma_start(out=poswT,
                   in_=_ap(io["pos_wT"], 0, [[F, DIM], [DIM * F, H], [1, F]]))

    # ---------------------------------------------------------------- helpers
    def ln_scale(xt, T, tag):
        """xt: list of 8 [128,T] f32 SBUF tiles (feature-major).
        Returns (xsc bf16 tiles, negmur bf16 [1,T])."""
        ps_sum = psum.tile([1, T], FP32, tag="ps_stat", bufs=2)
        ps_sq = psum.tile([1, T], FP32, tag="ps_stat", bufs=2)
        for k in range(8):
            xb = sb.tile([P, T], BF, tag="lnxb", bufs=2)
            act.copy(xb, xt[k])
            pe.matmul(ps_sum, ones_col, xb,
                      start=(k == 0), stop=(k == 7))
        for k in range(8):
            sq = sb.tile([P, T], BF, tag="lnsq", bufs=2)
            act.square(sq, xt[k])
            pe.matmul(ps_sq, ones_col, sq,
                      start=(k == 0), stop=(k == 7))
        mu = sb.tile([1, T], FP32, tag="lnstat", bufs=8)
        act.mul(mu, ps_sum, 1.0 / D)
        m2 = sb.tile([1, T], FP32, tag="lnstat", bufs=8)
        act.mul(m2, ps_sq, 1.0 / D)
        ve = sb.tile([1, T], FP32, tag="lnstat", bufs=8)
        vec.tensor_mul(ve, mu, mu)
        vec.tensor_sub(ve, m2, ve)
        vec.tensor_scalar_add(ve, ve, EPS)
        rinv = sb.tile([1, T], FP32, tag="lnstat", bufs=8)
        vec.reciprocal(rinv, ve)
        r = sb.tile([1, T], FP32, tag="lnstat", bufs=8)
        act.sqrt(r, rinv)
        mr = sb.tile([1, T], FP32, tag="lnstat", bufs=8)
        vec.tensor_mul(mr, mu, r)
        negmur = sb.tile([1, T], BF, tag="negmur", bufs=4)
        act.mul(negmur, mr, -1.0)
        rb = sb.tile([1, T], BF, tag="lnstatb", bufs=2)
        act.copy(rb, r)
        ps_b = psum.tile([P, T], FP32, tag="ps_bc", bufs=1)
        pe.matmul(ps_b, ones_row, rb, start=True, stop=True)
        r_bc = sb.tile([P, T], FP32, tag="r_bc", bufs=2)
        vec.tensor_copy(r_bc, ps_b)
        xsc = []
        for k in range(8):
            o = sb.tile([P, T], BF, tag=f"xsc_{tag}", bufs=8)
            vec.tensor_mul(o, xt[k], r_bc)
            xsc.append(o)
        return xsc, negmur

    def load_feature_major(handle, T):
        xt = []
        for k in range(8):
            t = sb.tile([P, T], FP32, tag="xf32", bufs=8)
            sync.dma_start(out=t, in_=handle.ap()[k * P:(k + 1) * P, :])
            xt.append(t)
        return xt

    def load_bias_cols(name, nm):
        t = consts.tile([P, nm], FP32, tag=f"bias_{name}")
        sync.dma_start(out=t, in_=_ap(io[name], 0, [[1, P], [P, nm]]))
        return t

    def ffn(i, mv, negmur, emit):
        """mv: 8 bf16 [128,S] moving tiles; emit(m, psum) consumes mm2 out."""
        w1t = []
        for k in range(8):
            t = sb.tile([P, HIDL], BF, tag="w1", bufs=8)
            sync.dma_start(out=t, in_=io[f"w1_{i}"].ap()[k * P:(k + 1) * P, :])
            w1t.append(t)
        w1sum = consts.tile([1, HIDL], BF, tag=f"w1sum{i}")
        sync.dma_start(out=w1sum, in_=io[f"w1sum_{i}"].ap())
        b1 = load_bias_cols(f"b1_{i}", 8)
        cc_in = dram.tile([HIDL, S], BF, tag="cc_h", bufs=2)
        hg = dram.tile([HID, S], BF, tag="hg", bufs=2)
        for m in range(8):
            ps = psum.tile([P, S], FP32, tag="ps_mm", bufs=2)
            for k in range(8):
                pe.matmul(ps, w1t[k][:, m * P:(m + 1) * P], mv[k],
                          start=(k == 0), stop=False)
            pe.matmul(ps, w1sum[:, m * P:(m + 1) * P], negmur,
                      start=False, stop=True)
            h = sb.tile([P, S], BF, tag="h", bufs=8)
            vec.tensor_scalar(h, ps, b1[:, m:m + 1], 0.0, op0=ALU.add, op1=ALU.max)
            sync.dma_start(out=cc_in[m * P:(m + 1) * P, :], in_=h)
        gps.collective_compute("AllGather", ALU.bypass, replica_groups=RG,
                               ins=[cc_in.opt()], outs=[hg.opt()])
        w2t, hgt = [], []
        for k in range(32):
            t = sb.tile([P, OUTD_S[i]], BF, tag="w2", bufs=32)
            sync.dma_start(out=t, in_=io[f"w2_{i}"].ap()[k * P:(k + 1) * P, :])
            w2t.append(t)
            t2 = sb.tile([P, S], BF, tag="hrem", bufs=32)
            sync.dma_start(out=t2, in_=hg[k * P:(k + 1) * P, :])
            hgt.append(t2)
        for m in range(OUTD_S[i] // P):
            ps = psum.tile([P, S], FP32, tag="ps_mm", bufs=2)
            for k in range(32):
                pe.matmul(ps, w2t[k][:, m * P:(m + 1) * P], hgt[k],
                          start=(k == 0), stop=(k == 31))
            emit(m, ps)

    def softmax_av(s_src, w_d, kv_handle, kvrow, a_pairs, nheads=H, do_av=True):
        """s_src(j) -> (psum_or_sbuf [128,S] scores for head pair j).
        Produces a_pairs[j] = psum [128(2h*64d), TLOC] attention outputs."""
        for j in range(nheads // 2):
            s_t = s_src(j)
            e = sb.tile([P, S], BF, tag="e", bufs=4)
            act.activation(e, s_t, AF.Exp)
            z = sb.tile([P, 1], FP32, tag="z", bufs=4)
            vec.reduce_sum(z, e, axis=mybir.AxisListType.X)
            rz = sb.tile([P, 1], FP32, tag="rz", bufs=4)
            vec.reciprocal(rz, z)
            en = sb.tile([P, S], BF, tag="en", bufs=4)
            vec.tensor_scalar_mul(en, e, rz)
            sync.dma_start(out=w_d[2 * j:2 * j + 2, :, :], in_=en)
        if not do_av:
            return
        for j in range(nheads // 2):
            a_ps = psum.tile([P, TLOC], FP32, tag="ps_attn", bufs=3)
            for hh in range(2):
                h = 2 * j + hh
                for kc in range(2):
                    wt = sb.tile([P, TLOC], BF, tag="wt", bufs=4)
                    vt = sb.tile([P, DIM], BF, tag="vt", bufs=4)
                    if NOTR:
                        sync.dma_start(out=wt, in_=_ap(
                            w_d, h * TLOC * S, [[TLOC, P], [1, TLOC]]))
                        sync.dma_start(out=vt, in_=_ap(
                            kv_handle, kvrow(h) * S, [[S, P], [1, DIM]]))
                    else:
                        sync.dma_start(out=wt, in_=w_d[h, :, kc * P:(kc + 1) * P],
                                       transpose=True)
                        sync.dma_start(
                            out=vt,
                            in_=kv_handle[kvrow(h):kvrow(h) + DIM, kc * P:(kc + 1) * P],
                            transpose=True)
                    pe.matmul(a_ps[hh * DIM:(hh + 1) * DIM, :], vt, wt,
                              start=(kc == 0), stop=(kc == 1))
            a_pairs.append(a_ps)

    # =================== LN0 + block0 ===================
    xt0 = load_feature_major(io["xT"], S)
    xsc0, negmur0 = ln_scale(xt0, S, "ln0")
    b2_0 = load_bias_cols("b2_0", 6)
    rankhot = consts.tile([P, G], FP32, tag="rankhot")
    sync.dma_start(out=rankhot, in_=io["rankhot"].ap())
    # q rides a ReduceScatter (one-hot-masked replicas -> pure selection, so
    # every core's own q-token slice lands at a fixed address); k/v AllGather.
    # cc_qrs: [shard j(receiver), r'(sender block), (hl,d), t(recv tokens)]
    cc_qrs = dram.tile([G, G, G * DIM, TLOC], BF, tag="cc_qrs")
    qg = dram.tile([D, TLOC], BF, tag="qg")
    cc_kv0 = dram.tile([2 * G * DIM, S], BF, tag="cc_kv0")     # [512,256]
    kvg0 = dram.tile([2 * D, S], BF, tag="kvg0")
    QRS_J = G * G * DIM * TLOC      # shard stride (elems)
    QRS_R = G * DIM * TLOC          # sender-block stride

    def emit0(m, ps):
        o = sb.tile([P, S], BF, tag="qkvband", bufs=4)
        for j in range(2):
            colbase = m * P + j * 64
            hl, part = colbase // 192, (colbase % 192) // 64   # part:0=q 1=k 2=v
            band = slice(j * 64, (j + 1) * 64)
            if part == 0:
                for rp_ in range(G):
                    oq = sb.tile([P, S], BF, tag="qband", bufs=4)
                    vec.tensor_scalar(oq[band, :], ps[band, :],
                                      b2_0[band, m:m + 1], rankhot[band, rp_:rp_ + 1],
                                      op0=ALU.add, op1=ALU.mult)
                    sync.dma_start(
                        out=_ap(cc_qrs, rp_ * QRS_R + hl * DIM * TLOC,
                                [[TLOC, DIM], [QRS_J, G], [1, TLOC]]),
                        in_=oq[band, :])
            else:
                sc = 0.125 if part == 1 else 1.0
                vec.tensor_scalar(o[band, :], ps[band, :],
                                  b2_0[band, m:m + 1], sc, op0=ALU.add, op1=ALU.mult)
                row = hl * 2 * DIM + (part - 1) * DIM
                sync.dma_start(out=cc_kv0[row:row + DIM, :], in_=o[band, :])

    ffn(0, xsc0, negmur0, emit0)
    gps.collective_compute("ReduceScatter", ALU.add, replica_groups=RG,
                           ins=[cc_qrs.opt()], outs=[qg.opt()])
    gps.collective_compute("AllGather", ALU.bypass, replica_groups=RG,
                           ins=[cc_kv0.opt()], outs=[kvg0.opt()])

    if KMODE < 2:
        ctx.close()
        return
    # =================== LN2 + block2 (cross kv) ===================
    xt2 = load_feature_major(io["vT"], S)
    xsc2, negmur2 = ln_scale(xt2, S, "ln2")
    b2_2 = load_bias_cols("b2_2", 4)
    cc_kv2 = dram.tile([2 * G * DIM, S], BF, tag="cc_kv2")
    kvg2 = dram.tile([2 * D, S], BF, tag="kvg2")

    def emit2(m, ps):
        o = sb.tile([P, S], BF, tag="qkvband", bufs=4)
        for j in range(2):
            colbase = m * P + j * 64
            hl, part = colbase // 128, (colbase % 128) // 64   # 0=k 1=v
            sc = 0.125 if part == 0 else 1.0
            band = slice(j * 64, (j + 1) * 64)
            vec.tensor_scalar(o[band, :], ps[band, :],
                              b2_2[band, m:m + 1], sc, op0=ALU.add, op1=ALU.mult)
            row = hl * 2 * DIM + part * DIM
            sync.dma_start(out=cc_kv2[row:row + DIM, :], in_=o[band, :])

    ffn(2, xsc2, negmur2, emit2)
    gps.collective_compute("AllGather", ALU.bypass, replica_groups=RG,
                           ins=[cc_kv2.opt()], outs=[kvg2.opt()])

    if KMODE < 3:
        ctx.close()
        return
    # =================== self-attention ===================
    # qp^T per head -> qp_ext [65(f), 16(h), 64(q)] with ones row for mask
    qht = []
    for h in range(H):
        t = sb.tile([DIM, TLOC], BF, tag="qh", bufs=16)
        sync.dma_start(out=t, in_=qg[h * DIM:(h + 1) * DIM, :])
        qht.append(t)
    qp_ext = sb.tile([F + 1, H, TLOC], BF, tag="qp_ext", bufs=1)
    vec.memset(qp_ext[F:F + 1, :, :], 1.0)
    for h in range(H):
        qp_ps = psum.tile([F, TLOC], FP32, tag="ps_attn", bufs=3)
        pe.matmul(qp_ps, poswT[:, h, :], qht[h], start=True, stop=True)
        vec.tensor_copy(qp_ext[0:F, h, :], qp_ps)
    # bias per q: [16(h), S] = qp_ext[:,:,q].T @ rpT_ext[q]
    bias_d = dram.tile([TLOC, H, S], BF, tag="bias_d")
    for q0 in range(0, TLOC, 4):
        bp = psum.tile([P, S], FP32, tag="ps_attn", bufs=3)
        for qi in range(4):
            rpt = sb.tile([F + 1, S], BF, tag="rpt", bufs=6)
            sync.dma_start(out=rpt, in_=io["rpT_ext"].ap()[q0 + qi, :, :])
            pe.matmul(bp[qi * 32:qi * 32 + H, :], qp_ext[:, :, q0 + qi], rpt,
                      start=True, stop=True, tile_position=(0, qi * 32))
        bsb = sb.tile([P, S], BF, tag="bsb", bufs=4)
        for qi in range(4):
            rows = slice(qi * 32, qi * 32 + H)
            vec.tensor_copy(bsb[rows, :], bp[rows, :])
            sync.dma_start(out=bias_d[q0 + qi, :, :], in_=bsb[rows, :])
    if KMODE < 4:
        ctx.close()
        return
    # scores + bias -> softmax -> AV
    w_d0 = dram.tile([H, TLOC, S], BF, tag="w_d0")
    a0_pairs = []

    def s_src0(j):
        s_ps = psum.tile([P, S], FP32, tag="ps_attn", bufs=3)
        for hh in range(2):
            h = 2 * j + hh
            kt = sb.tile([DIM, S], BF, tag="kh", bufs=4)
            sync.dma_start(out=kt, in_=kvg0[h * 2 * DIM:h * 2 * DIM + DIM, :])
            pe.matmul(s_ps[hh * 64:(hh + 1) * 64, :], qht[h], kt,
                      start=True, stop=True)
        bp = sb.tile([P, S], BF, tag="bp", bufs=4)
        sync.dma_start(
            out=bp, in_=_ap(bias_d.tensor, 2 * j * S,
                            [[S, 2], [H * S, TLOC], [1, S]]))
        s_sb = sb.tile([P, S], FP32, tag="s_sb", bufs=4)
        vec.tensor_add(s_sb, s_ps, bp)
        return s_sb

    softmax_av(s_src0, w_d0, kvg0, lambda h: h * 2 * DIM + DIM, a0_pairs,
               do_av=(KMODE >= 5))

    if KMODE < 5:
        ctx.close()
        return
    # x1 = queries_slice + a0 ; LN1 -> AllGather (with negmur row)
    x1t = []
    for k in range(8):
        xs = sb.tile([P, TLOC], FP32, tag="xslice", bufs=8)
        sync.dma_start(out=xs, in_=io["xTslice"].ap()[k * P:(k + 1) * P, :])
        x1 = sb.tile([P, TLOC], FP32, tag="x1", bufs=8)
        vec.tensor_add(x1, xs, a0_pairs[k])
        x1t.append(x1)
    xsc1, negmur1 = ln_scale(x1t, TLOC, "ln1")
    cc_x1 = dram.tile([D + 1, TLOC], BF, tag="cc_x1")
    x1ng = dram.tile([G, D + 1, TLOC], BF, tag="x1ng")
    for k in range(8):
        sync.dma_start(out=cc_x1[k * P:(k + 1) * P, :], in_=xsc1[k])
    sync.dma_start(out=cc_x1[D:D + 1, :], in_=negmur1)
    gps.collective_compute("AllGather", ALU.bypass, replica_groups=RG,
                           ins=[cc_x1.opt()], outs=[x1ng.opt()])

    if KMODE < 6:
        ctx.close()
        return
    # =================== block1 (y = cross-attn queries) ===================
    mv1 = []
    st1 = (D + 1) * TLOC
    for k in range(8):
        t = sb.tile([P, S], BF, tag="agin", bufs=8)
        sync.dma_start(out=t, in_=_ap(x1ng.tensor, k * P * TLOC,
                                      [[TLOC, P], [st1, G], [1, TLOC]]))
        mv1.append(t)
    negmur1f = sb.tile([1, S], BF, tag="negmur", bufs=4)
    sync.dma_start(out=negmur1f, in_=_ap(x1ng.tensor, D * TLOC,
                                         [[1, 1], [st1, G], [1, TLOC]]))
    b2_1 = load_bias_cols("b2_1", 2)
    cc_yrs = dram.tile([G, G, G * DIM, TLOC], BF, tag="cc_yrs")
    yg = dram.tile([D, TLOC], BF, tag="yg")

    def emit1(m, ps):
        for rp_ in range(G):
            o = sb.tile([P, S], BF, tag="yband", bufs=4)
            vec.tensor_scalar(o, ps, b2_1[:, m:m + 1], rankhot[:, rp_:rp_ + 1],
                              op0=ALU.add, op1=ALU.mult)
            sync.dma_start(
                out=_ap(cc_yrs, rp_ * QRS_R + m * P * TLOC,
                        [[TLOC, P], [QRS_J, G], [1, TLOC]]),
                in_=o)

    ffn(1, mv1, negmur1f, emit1)
    gps.collective_compute("ReduceScatter", ALU.add, replica_groups=RG,
                           ins=[cc_yrs.opt()], outs=[yg.opt()])

    if KMODE < 7:
        ctx.close()
        return
    # =================== cross-attention ===================
    w_d1 = dram.tile([H, TLOC, S], BF, tag="w_d1")
    a1_pairs = []

    def s_src1(j):
        s_ps = psum.tile([P, S], FP32, tag="ps_attn", bufs=3)
        for hh in range(2):
            h = 2 * j + hh
            qt = sb.tile([DIM, TLOC], BF, tag="yh", bufs=4)
            sync.dma_start(out=qt, in_=yg[h * DIM:(h + 1) * DIM, :])
            kt = sb.tile([DIM, S], BF, tag="kh", bufs=4)
            sync.dma_start(out=kt, in_=kvg2[h * 2 * DIM:h * 2 * DIM + DIM, :])
            pe.matmul(s_ps[hh * 64:(hh + 1) * 64, :], qt, kt,
                      start=True, stop=True)
        return s_ps

    softmax_av(s_src1, w_d1, kvg2, lambda h: h * 2 * DIM + DIM, a1_pairs)

    # x2 = x1 + a1 -> output + LN3 -> AllGather
    x2t = []
    for k in range(8):
        x2 = sb.tile([P, TLOC], FP32, tag="x2", bufs=8)
        vec.tensor_add(x2, x1t[k], a1_pairs[k])
        x2t.append(x2)
        sync.dma_start(out=io["x2T_out"].ap()[k * P:(k + 1) * P, :], in_=x2)
    xsc3, negmur3 = ln_scale(x2t, TLOC, "ln3")
    cc_x3 = dram.tile([D + 1, TLOC], BF, tag="cc_x3")
    x3ng = dram.tile([G, D + 1, TLOC], BF, tag="x3ng")
    for k in range(8):
        sync.dma_start(out=cc_x3[k * P:(k + 1) * P, :], in_=xsc3[k])
    sync.dma_start(out=cc_x3[D:D + 1, :], in_=negmur3)
    gps.collective_compute("AllGather", ALU.bypass, replica_groups=RG,
                           ins=[cc_x3.opt()], outs=[x3ng.opt()])

    if KMODE < 8:
        ctx.close()
        return
    # =================== block3 ===================
    mv3 = []
    for k in range(8):
        t = sb.tile([P, S], BF, tag="agin", bufs=8)
        sync.dma_start(out=t, in_=_ap(x3ng.tensor, k * P * TLOC,
                                      [[TLOC, P], [st1, G], [1, TLOC]]))
        mv3.append(t)
    negmur3f = sb.tile([1, S], BF, tag="negmur", bufs=4)
    sync.dma_start(out=negmur3f, in_=_ap(x3ng.tensor, D * TLOC,
                                         [[1, 1], [st1, G], [1, TLOC]]))
    b2_3 = load_bias_cols("b2_3", 2)

    def emit3(m, ps):
        o = sb.tile([P, S], FP32, tag="o3", bufs=2)
        vec.tensor_scalar_add(o, ps, b2_3[:, m:m + 1])
        sync.dma_start(out=io["o3T_out"].ap()[m * P:(m + 1) * P, :], in_=o)

    ffn(3, mv3, negmur3f, emit3)
    ctx.close()


# ------------------------------------------------------------------- runner
def kernel(**inputs) -> np.ndarray:
    if "nc" not in _CACHE:
        _CACHE["nc"] = _build_nc()
    nc = _CACHE["nc"]
    in_maps = _prep_in_maps(inputs)
    res = run_bass_kernel_spmd(nc, in_maps, core_ids=list(range(8)))
    out = np.zeros((B, S, D), np.float32)
    for g in range(B):
        x2T = np.concatenate(
            [np.asarray(res.results[g * G + s]["x2T_out"]) for s in range(G)], axis=1)
        o3T = np.concatenate(
            [np.asarray(res.results[g * G + s]["o3T_out"]) for s in range(G)], axis=0)
        out[g] = (x2T + o3T).T
    return out
